# revision 16
# baseline (speedup 1.0000x reference)
"""Deformable conv (3x3 + offset conv) for Trainium2, 8 cores, data parallel.

Core k: sample k//2, row block (k%2)*48..+48 (4608 px). Per-core pipeline:
  A. offset conv (3x3, C=256 -> 18) in 12 sub-chunks of 384 cols; output
     channels host-reordered to [dy0..8, dx0..8] and written into partition
     group g*32 of offb[128, 1152] (tap-major, pixels in free dim).
  B. PE-transpose offb -> doff[p, t, 18] (pixel-major) for the weight math.
  C1. DVE weight math (pixel-major): quad-anchor bilinear corner weights with
      zero-pad validity and the y/x==-1 slot-swap trick.
  C2. DVE index math (tap-major): quad anchor idx = clip(y0)*96+clip(x0);
      free-dim restride makes the SWDGE wrapped-16 fold 16 contiguous DMAs.
  E. per (stage of 512 px, tap): ONE gather descriptor per (pixel,tap)
     fetches a 2-entry window of the quad token table (= all 4 bilinear
     corners, 2KB); DVE 4-term FMA with per-partition scalar weights; PE
     transpose to channel layout; matmul accumulating over (c,tap) into PSUM.
"""

import dataclasses

import numpy as np

import concourse.bacc as bacc
import concourse.bass as bass
import concourse.mybir as mybir
import concourse.tile as tile
from concourse import bass_utils, masks
from concourse.mybir import ActivationFunctionType as Act
from concourse.mybir import AluOpType as Op

P = 128
B, C, H, W, O = 4, 256, 96, 96, 256
K = 3
K2 = 9
NCORES = 8
ROWS = 48                      # output rows per core
NPIX = ROWS * W                # 4608
NSTAGE = 9                     # gather stages of 512 px
SPX = 512
TPS = 4                        # 128-px tiles per stage
NTILE = 36
NGRP = 3                       # conv/fold groups of 1536 px at partition g*32
GP = 1536
GTL = GP // P                  # 128-px tiles per group = 12
GCOL = GP // 16                # idx cols per (group, tap) = 96
SCOL = SPX // 16               # idx cols per (stage, tap) = 32
PADH, PADW = ROWS + 2, W + 2   # 50, 98
NTOK = H * W                   # 9216
NQENT = NTOK + 4               # quad table entries (pad for idx+1 overhang)
CONV_ROWS = 4                  # conv sub-chunk = 4 rows = 384 cols
NSUB = 12                      # conv sub-chunks (4 per group)
NSWQ = 4                       # SWDGE queues (alternate to pipeline ant/DMA)
BF = mybir.dt.bfloat16
F32 = mybir.dt.float32
I16 = mybir.dt.int16
MAGIC = 8388608.0

_BUILT = {}


def _emit(tc, nc, io):
    qt, xc, wofl, boff, wdcl, bdc, pyb, pxb, pypxbT, out = io

    with (
        tc.tile_pool(name="const", bufs=1) as cpool,
        tc.tile_pool(name="sbig", bufs=1) as spool,
    ):
        ident_bf = cpool.tile([P, P], BF, tag="idbf", name="idbf")
        ident_f = cpool.tile([P, P], F32, tag="idf", name="idf")
        masks.make_identity(nc, ident_bf[:])
        masks.make_identity(nc, ident_f[:])

        xc_sb = spool.tile([P, 2, PADH * PADW], BF, tag="xc", name="xc")
        wofl_sb = spool.tile([P, 2, K2, 18], BF, tag="wofl", name="wofl")
        wdcl_sb = spool.tile([P, K2, 2, 2, P], BF, tag="wdcl", name="wdcl")
        boff_sb = spool.tile([18, 1], F32, tag="boff", name="boff")
        bdc_sb = spool.tile([P, 2], F32, tag="bdc", name="bdc")
        pyb_sb = spool.tile([P, NTILE, K2], F32, tag="pyb", name="pyb")
        pxb_sb = spool.tile([P, NTILE, K2], F32, tag="pxb", name="pxb")
        pypxbT_sb = spool.tile([P, GP], F32, tag="pypxbT", name="pypxbT")
        offb = spool.tile([P, GP], F32, tag="offb", name="offb")
        doff = spool.tile([P, NTILE, 18], F32, tag="doff", name="doff")
        wt = spool.tile([P, 4, NTILE, K2], F32, tag="wt", name="wt")
        idxI = spool.tile([P, 16, GCOL], I16, tag="idxI", name="idxI")
        idxC = spool.tile([K2 * NGRP, 16, GCOL], I16, tag="idxC", name="idxC")
        twrap = spool.tile([P, NGRP * K2 * GCOL], I16, tag="twrap", name="twrap")

        nc.sync.dma_start(xc_sb[:], xc)
        nc.sync.dma_start(wofl_sb[:], wofl)
        nc.sync.dma_start(wdcl_sb[:], wdcl)
        nc.sync.dma_start(boff_sb[:], boff)
        nc.sync.dma_start(bdc_sb[:], bdc)
        nc.sync.dma_start(pyb_sb[:], pyb)
        nc.sync.dma_start(pxb_sb[:], pxb)
        nc.sync.dma_start(pypxbT_sb[:], pypxbT)

        # ---- A: offset conv, 12 sub-chunks of 384 cols; out regrouped ----
        with tc.tile_pool(name="psA", bufs=2, space="PSUM") as psa:
            for sc in range(NSUB):
                g, sub = sc // 4, sc % 4
                ncols = CONV_ROWS * W  # 384
                ps = psa.tile([18, ncols], F32, tag="psoff", name="psoff")
                n_mm = 2 * K2
                mm = 0
                xcf = xc_sb[:]
                for chalf in range(2):
                    for tap in range(K2):
                        ti, tj = tap // K, tap % K
                        rhs = dataclasses.replace(
                            xcf,
                            ap=[
                                [xcf.ap[0][0], P],
                                [PADW, CONV_ROWS],
                                [1, W],
                            ],
                            offset=xcf.offset
                            + chalf * (PADH * PADW)
                            + ((sc * CONV_ROWS + ti) * PADW + tj),
                        )
                        nc.tensor.matmul(
                            ps[:],
                            wofl_sb[:, chalf, tap],
                            rhs,
                            start=(mm == 0),
                            stop=(mm == n_mm - 1),
                        )
                        mm += 1
                # group g at partitions [g*32, g*32+18):
                # rows +0..8 = dy taps, +9..17 = dx taps
                nc.scalar.activation(
                    offb[g * 32 : g * 32 + 18, sub * ncols : (sub + 1) * ncols],
                    ps[:],
                    Act.Identity,
                    bias=boff_sb[:],
                )

        # ---- B: transpose offsets to pixel layout doff[p, t, (dy9,dx9)] ----
        with tc.tile_pool(name="psB", bufs=4, space="PSUM") as psb:
            for tl in range(GTL):
                pt = psb.tile([P, P], F32, tag="pofft", name="pofft")
                nc.tensor.transpose(
                    pt[:], offb[:, tl * P : (tl + 1) * P], ident_f[:]
                )
                for g in range(NGRP):
                    nc.scalar.copy(
                        doff[:, g * GTL + tl, :],
                        pt[:, g * 32 : g * 32 + 18],
                    )

        # ---- C1: pixel-major weight math (DVE over [128, 36*9]) ----
        with tc.tile_pool(name="scr", bufs=1) as scr:
            sh = [P, NTILE, K2]

            def tmp(tag):
                return scr.tile(sh, F32, tag=tag, name=tag)

            dy = doff[:, :, 0:9]
            dx = doff[:, :, 9:18]
            py = tmp("py")
            px = tmp("px")
            # pyb/pxb carry the +16 shift already
            nc.vector.tensor_tensor(py[:], pyb_sb[:], dy, Op.add)
            nc.vector.tensor_tensor(px[:], pxb_sb[:], dx, Op.add)
            y0 = tmp("y0")
            x0 = tmp("x0")
            nc.vector.tensor_scalar(y0[:], py[:], -0.4999999, None, Op.add)
            nc.vector.tensor_scalar(y0[:], y0[:], MAGIC, -MAGIC, Op.add, Op.add)
            nc.vector.tensor_scalar(x0[:], px[:], -0.4999999, None, Op.add)
            nc.vector.tensor_scalar(x0[:], x0[:], MAGIC, -MAGIC, Op.add, Op.add)
            ly = tmp("ly")
            lx = tmp("lx")
            nc.vector.tensor_tensor(ly[:], py[:], y0[:], Op.subtract)
            nc.vector.tensor_tensor(lx[:], px[:], x0[:], Op.subtract)

            ta_ = tmp("ta")
            tb_ = tmp("tb")
            tc_ = tmp("tc")
            td_ = tmp("td")

            def wab(f0, frac, wA, wB):
                # slot weights for a quad axis (coords shifted +16):
                # wA = (1-frac)*[16<=f0<=111] + frac*[f0==15]
                # wB = frac*[16<=f0<=110]
                nc.vector.tensor_scalar(ta_[:], f0[:], 16.0, None, Op.is_ge)
                nc.vector.tensor_scalar(tb_[:], f0[:], 111.0, None, Op.is_le)
                nc.vector.tensor_tensor(tb_[:], ta_[:], tb_[:], Op.mult)
                nc.vector.tensor_scalar(tc_[:], frac[:], -1.0, 1.0, Op.mult, Op.add)
                nc.vector.tensor_tensor(tb_[:], tc_[:], tb_[:], Op.mult)
                nc.vector.tensor_scalar(td_[:], f0[:], 15.0, None, Op.is_equal)
                nc.vector.tensor_tensor(td_[:], frac[:], td_[:], Op.mult)
                nc.vector.tensor_tensor(wA[:], tb_[:], td_[:], Op.add)
                nc.vector.tensor_scalar(tc_[:], f0[:], 110.0, None, Op.is_le)
                nc.vector.tensor_tensor(tc_[:], ta_[:], tc_[:], Op.mult)
                nc.vector.tensor_tensor(wB[:], frac[:], tc_[:], Op.mult)

            wyA = tmp("wyA")
            wyB = tmp("wyB")
            wxA = tmp("wxA")
            wxB = tmp("wxB")
            wab(y0, ly, wyA, wyB)
            wab(x0, lx, wxA, wxB)
            # corner cr = b*2 + a (b = x entry, a = y row): weight wy_a * wx_b
            nc.vector.tensor_tensor(wt[:, 0], wyA[:], wxA[:], Op.mult)
            nc.vector.tensor_tensor(wt[:, 1], wyB[:], wxA[:], Op.mult)
            nc.vector.tensor_tensor(wt[:, 2], wyA[:], wxB[:], Op.mult)
            nc.vector.tensor_tensor(wt[:, 3], wyB[:], wxB[:], Op.mult)

            # ---- C2: tap-major index math ([128, 1152]) ----
            pypx = scr.tile([P, GP], F32, tag="pypx", name="pypx")
            nc.vector.tensor_tensor(pypx[:], offb[:], pypxbT_sb[:], Op.add)
            nc.vector.tensor_scalar(pypx[:], pypx[:], -0.4999999, None, Op.add)
            nc.vector.tensor_scalar(
                pypx[:], pypx[:], MAGIC, -MAGIC, Op.add, Op.add
            )
            nc.vector.tensor_scalar(
                pypx[:], pypx[:], 16.0, 111.0, Op.max, Op.min
            )  # quad anchor (shifted +16)
            # partition-shift by 9 via DMA so the y*96+x mix is base-aligned
            anchX = scr.tile([P, GP], F32, tag="anchX", name="anchX")
            nc.sync.dma_start(anchX[0:119, :], pypx[9:128, :])
            idxf = scr.tile([P, GP], F32, tag="idxf", name="idxf")
            nc.vector.scalar_tensor_tensor(
                idxf[0:74, :], pypx[0:74, :], 96.0, anchX[0:74, :],
                Op.mult, Op.add,
            )
            nc.vector.tensor_scalar(
                idxf[0:74, :], idxf[0:74, :], -1552.0, None, Op.add
            )
            # convert + restride: idxI[g*32+tap, r, col] <- idxf[g*32+tap, col*16+r]
            for g in range(NGRP):
                src = idxf[g * 32 : g * 32 + 9, :]
                src = dataclasses.replace(
                    src, ap=[src.ap[0], [1, 16], [16, GCOL]]
                )
                nc.vector.tensor_copy(
                    idxI[g * 32 : g * 32 + 9, :, :], src
                )

        # ---- D: fold into wrapped-16 layout (contiguous DMAs) ----
        # hop 1: compact to contiguous rows: idxC[g*9+tap, r, col]
        for g in range(NGRP):
            nc.sync.dma_start(
                idxC[g * K2 : (g + 1) * K2, :, :],
                idxI[g * 32 : g * 32 + K2, :, :],
            )
        # hop 2: twrap[r, (g*9+tap)*GCOL + col] <- idxC[g*9+tap, r, col]
        cfl = idxC[:]
        pitch_c = cfl.ap[0][0]
        for r in range(16):
            src = dataclasses.replace(
                cfl,
                ap=[[pitch_c, K2 * NGRP], [1, GCOL]],
                offset=cfl.offset + r * GCOL,
            )
            nc.sync.dma_start(twrap[r : r + 1, :], src)
        for g in range(1, 8):
            nc.sync.dma_start(twrap[16 * g : 16 * (g + 1), :], twrap[0:16, :])

        # ---- E: main loop ----
        with (
            tc.tile_pool(name="gpool", bufs=4) as gpool,
            tc.tile_pool(name="vpool", bufs=4) as vpool,
            tc.tile_pool(name="upool", bufs=8) as upool,
            tc.tile_pool(name="rpool", bufs=2) as rpool,
            tc.tile_pool(name="opool", bufs=2) as opool,
            tc.tile_pool(name="psT", bufs=4, space="PSUM") as pst,
            tc.tile_pool(name="psO", bufs=2, space="PSUM") as pso,
        ):
            # quad-table window view: [NQENT-1 entries, 1024] stride 512
            qt_win = dataclasses.replace(
                qt, ap=[[2 * C, NQENT - 1], [1, 4 * C]], offset=0
            )
            for s in range(NSTAGE):
                po = pso.tile([P, 2, SPX], F32, tag="po", name="po")
                for tap in range(K2):
                    g_t = gpool.tile([P, TPS, 4 * C], BF, tag="g", name="g")
                    gq, qq = s // 3, s % 3
                    base = (gq * K2 + tap) * GCOL + qq * SCOL
                    idxs = twrap[:, base : base + SCOL]
                    nc.gpsimd.dma_gather(
                        g_t[:],
                        qt_win,
                        idxs,
                        SPX,
                        SPX,
                        elem_size=4 * C,
                        elem_step=2 * C,
                        single_packet=False,
                        queue_num=(tap % NSWQ),
                    )
                    rst = rpool.tile([P, 2, SPX], BF, tag="rst", name="rst")
                    us = []
                    for tl in range(TPS):
                        t = s * TPS + tl
                        u = upool.tile([P, C], BF, tag="u", name="u")
                        w0 = wt[:, 0, t, tap : tap + 1]
                        # corner 0 on ScalarE (out = in*scale), rest on DVE;
                        # all four hoisted so ScalarE runs ahead of the DVE chain
                        nc.scalar.activation(
                            u[:], g_t[:, tl, 0:256], Act.Identity, scale=w0
                        )
                        us.append(u)
                    for tl in range(TPS):
                        t = s * TPS + tl
                        v = vpool.tile([P, C], BF, tag="v", name="v")
                        u = us[tl]
                        w1 = wt[:, 1, t, tap : tap + 1]
                        w2 = wt[:, 2, t, tap : tap + 1]
                        w3 = wt[:, 3, t, tap : tap + 1]
                        nc.vector.scalar_tensor_tensor(
                            v[:], g_t[:, tl, 256:512], w1, u[:], Op.mult, Op.add
                        )
                        nc.vector.scalar_tensor_tensor(
                            v[:], g_t[:, tl, 512:768], w2, v[:], Op.mult, Op.add
                        )
                        nc.vector.scalar_tensor_tensor(
                            v[:], g_t[:, tl, 768:1024], w3, v[:], Op.mult, Op.add
                        )
                        ptr = pst.tile([P, 2, P], BF, tag="ptr", name="ptr")
                        nc.tensor.transpose(
                            ptr[:, 0, :], v[:, 0:P], ident_bf[:]
                        )
                        nc.tensor.transpose(
                            ptr[:, 1, :], v[:, P : 2 * P], ident_bf[:]
                        )
                        nc.scalar.copy(
                            rst[:, :, tl * P : (tl + 1) * P], ptr[:]
                        )
                    for chalf in range(2):
                        for oh in range(2):
                            nc.tensor.matmul(
                                po[:, oh],
                                wdcl_sb[:, tap, chalf, oh],
                                rst[:, chalf],
                                start=(tap == 0 and chalf == 0),
                                stop=(tap == K2 - 1 and chalf == 1),
                            )
                for oh in range(2):
                    ob = opool.tile([P, SPX], F32, tag="ob", name="ob")
                    nc.scalar.activation(
                        ob[:], po[:, oh], Act.Identity, bias=bdc_sb[:, oh : oh + 1]
                    )
                    nc.sync.dma_start(
                        out[oh, :, s * SPX : (s + 1) * SPX], ob[:]
                    )


def _build():
    if "nc" in _BUILT:
        return _BUILT["nc"]
    nc = bacc.Bacc(
        "TRN2",
        target_bir_lowering=False,
        debug=False,
        enable_asserts=False,
        num_devices=NCORES,
        num_swdge_queues=NSWQ,
    )
    qt = nc.dram_tensor("qt", [NQENT, 2 * C], BF, kind="ExternalInput").ap()
    xc = nc.dram_tensor("xc", [P, 2, PADH * PADW], BF, kind="ExternalInput").ap()
    wofl = nc.dram_tensor("wofl", [P, 2, K2, 18], BF, kind="ExternalInput").ap()
    boff = nc.dram_tensor("boff", [18, 1], F32, kind="ExternalInput").ap()
    wdcl = nc.dram_tensor("wdcl", [P, K2, 2, 2, P], BF, kind="ExternalInput").ap()
    bdc = nc.dram_tensor("bdc", [P, 2], F32, kind="ExternalInput").ap()
    pyb = nc.dram_tensor("pyb", [P, NTILE, K2], F32, kind="ExternalInput").ap()
    pxb = nc.dram_tensor("pxb", [P, NTILE, K2], F32, kind="ExternalInput").ap()
    pypxbT = nc.dram_tensor("pypxbT", [P, GP], F32, kind="ExternalInput").ap()
    out = nc.dram_tensor("out", [2, P, NPIX], F32, kind="ExternalOutput").ap()
    with tile.TileContext(nc) as tc:
        _emit(tc, nc, (qt, xc, wofl, boff, wdcl, bdc, pyb, pxb, pypxbT, out))
    nc.compile()
    _BUILT["nc"] = nc
    return nc


def _prep_core(k, x, w_off, b_off, w_dc, b_dc):
    b, half = k // 2, k % 2
    y0 = half * ROWS
    xs = x[b]  # [C,H,W] f32
    qt = np.zeros((NQENT, 2 * C), np.float32)
    qt[:NTOK, 0:C] = xs.transpose(1, 2, 0).reshape(NTOK, C)
    qt[: (H - 1) * W, C : 2 * C] = xs[:, 1:, :].transpose(1, 2, 0).reshape(
        (H - 1) * W, C
    )
    xc = np.zeros((C, PADH, PADW), np.float32)
    r0, r1 = max(0, y0 - 1), min(H, y0 + ROWS + 1)
    xc[:, (r0 - (y0 - 1)) : (r1 - (y0 - 1)), 1 : 1 + W] = xs[:, r0:r1, :]
    xc = xc.reshape(2, P, PADH * PADW).transpose(1, 0, 2)

    # conv output channels reordered to [dy0..8, dx0..8]
    perm = np.concatenate([np.arange(0, 18, 2), np.arange(1, 18, 2)])
    wofl = (
        w_off.reshape(2 * K2, 2, P, K2)   # [oc, chalf, c, tap]
        .transpose(2, 1, 3, 0)            # [c, chalf, tap, oc]
        [:, :, :, perm]
        .copy()
    )
    boff = b_off[perm].reshape(18, 1).astype(np.float32)
    wdcl = (
        w_dc.reshape(2, P, 2, P, K2)      # [oh, o, chalf, c, tap]
        .transpose(3, 4, 2, 0, 1)         # [c, tap, chalf, oh, o]
        .copy()
    )
    bdc = b_dc.reshape(2, P).transpose(1, 0).copy()

    pp = np.arange(NPIX)
    yg = y0 + pp // W
    xg = pp % W
    ti = (np.arange(K2) // K)[None, :]
    tj = (np.arange(K2) % K)[None, :]
    pyb = (yg[:, None] - 1 + ti + 16.0).astype(np.float32).reshape(NTILE, P, K2)
    pxb = (xg[:, None] - 1 + tj + 16.0).astype(np.float32).reshape(NTILE, P, K2)

    # tap-major base table [128, 1152]: row g*32 + yx*9 + tap
    pypxbT = np.zeros((P, GP), np.float32)
    fo = np.arange(GP)
    for g in range(NGRP):
        p = g * GP + fo
        for tap in range(K2):
            pypxbT[g * 32 + tap] = (y0 + p // W) - 1 + tap // K + 16.0
            pypxbT[g * 32 + 9 + tap] = (p % W) - 1 + tap % K + 16.0

    import ml_dtypes

    bf16 = ml_dtypes.bfloat16
    return {
        "qt": qt.astype(bf16),
        "xc": xc.astype(bf16),
        "wofl": wofl.astype(bf16),
        "boff": boff,
        "wdcl": wdcl.astype(bf16),
        "bdc": bdc.astype(np.float32),
        "pyb": pyb.transpose(1, 0, 2).copy(),
        "pxb": pxb.transpose(1, 0, 2).copy(),
        "pypxbT": pypxbT,
    }


def kernel(x, w_off, b_off, w_dc, b_dc, _trace=False):
    nc = _build()
    x = np.asarray(x, np.float32)
    w_off = np.asarray(w_off, np.float32)
    b_off = np.asarray(b_off, np.float32)
    w_dc = np.asarray(w_dc, np.float32)
    b_dc = np.asarray(b_dc, np.float32)
    in_maps = [
        _prep_core(k, x, w_off, b_off, w_dc, b_dc) for k in range(NCORES)
    ]
    res = bass_utils.run_bass_kernel_spmd(
        nc, in_maps, core_ids=list(range(NCORES)), trace=_trace
    )
    out = np.empty((B, O, H, W), np.float32)
    for k in range(NCORES):
        b, half = k // 2, k % 2
        o = res.results[k]["out"]  # [2,128,4608]
        out[b, :, half * ROWS : (half + 1) * ROWS, :] = o.reshape(
            O, ROWS, W
        )
    if _trace:
        return out, res
    return out


# revision 17
# speedup vs baseline: 1.0654x; 1.0654x over previous
"""Deformable conv (3x3 + offset conv) for Trainium2, 8 cores, data parallel.

Core k: sample k//2, row block (k%2)*48..+48 (4608 px). Per-core pipeline:
  A. offset conv (3x3, C=256 -> 18) in 12 sub-chunks of 384 cols; output
     channels host-reordered to [dy0..8, dx0..8] and written into partition
     group g*32 of offb[128, 1152] (tap-major, pixels in free dim).
  B. PE-transpose offb -> doff[p, t, 18] (pixel-major) for the weight math.
  C1. DVE weight math (pixel-major): quad-anchor bilinear corner weights with
      zero-pad validity and the y/x==-1 slot-swap trick.
  C2. DVE index math (tap-major): quad anchor idx = clip(y0)*96+clip(x0);
      free-dim restride makes the SWDGE wrapped-16 fold 16 contiguous DMAs.
  E. per (stage of 512 px, tap): ONE gather descriptor per (pixel,tap)
     fetches a 2-entry window of the quad token table (= all 4 bilinear
     corners, 2KB); DVE 4-term FMA with per-partition scalar weights; PE
     transpose to channel layout; matmul accumulating over (c,tap) into PSUM.
"""

import dataclasses

import numpy as np

import concourse.bacc as bacc
import concourse.bass as bass
import concourse.mybir as mybir
import concourse.tile as tile
from concourse import bass_utils, masks
from concourse.mybir import ActivationFunctionType as Act
from concourse.mybir import AluOpType as Op

P = 128
B, C, H, W, O = 4, 256, 96, 96, 256
K = 3
K2 = 9
NCORES = 8
ROWS = 48                      # output rows per core
NPIX = ROWS * W                # 4608
NSTAGE = 9                     # gather stages of 512 px
SPX = 512
TPS = 4                        # 128-px tiles per stage
NTILE = 36
NGRP = 3                       # conv/fold groups of 1536 px at partition g*32
GP = 1536
GTL = GP // P                  # 128-px tiles per group = 12
GCOL = GP // 16                # idx cols per (group, tap) = 96
SCOL = SPX // 16               # idx cols per (stage, tap) = 32
PADH, PADW = ROWS + 2, W + 2   # 50, 98
NTOK = H * W                   # 9216
NQENT = NTOK + 4               # quad table entries (pad for idx+1 overhang)
CONV_ROWS = 4                  # conv sub-chunk = 4 rows = 384 cols
NSUB = 12                      # conv sub-chunks (4 per group)
NSWQ = 4                       # SWDGE queues (alternate to pipeline ant/DMA)
BF = mybir.dt.bfloat16
F32 = mybir.dt.float32
I16 = mybir.dt.int16
MAGIC = 8388608.0

_BUILT = {}


def _emit(tc, nc, io):
    qt, xc, wofl, boff, wdcl, bdc, pyb, pxb, pypxbT, out = io

    with (
        tc.tile_pool(name="const", bufs=1) as cpool,
        tc.tile_pool(name="sbig", bufs=1) as spool,
    ):
        ident_bf = cpool.tile([P, P], BF, tag="idbf", name="idbf")
        ident_f = cpool.tile([P, P], F32, tag="idf", name="idf")
        masks.make_identity(nc, ident_bf[:])
        masks.make_identity(nc, ident_f[:])

        xc_sb = spool.tile([P, 2, PADH * PADW], BF, tag="xc", name="xc")
        wofl_sb = spool.tile([P, 2, K2, 18], BF, tag="wofl", name="wofl")
        wdcl_sb = spool.tile([P, K2, 2, 2, P], BF, tag="wdcl", name="wdcl")
        boff_sb = spool.tile([18, 1], F32, tag="boff", name="boff")
        bdc_sb = spool.tile([P, 2], F32, tag="bdc", name="bdc")
        pyb_sb = spool.tile([P, NTILE, K2], F32, tag="pyb", name="pyb")
        pxb_sb = spool.tile([P, NTILE, K2], F32, tag="pxb", name="pxb")
        pypxbT_sb = spool.tile([P, GP], F32, tag="pypxbT", name="pypxbT")
        offb = spool.tile([P, GP], F32, tag="offb", name="offb")
        doff = spool.tile([P, NTILE, 18], F32, tag="doff", name="doff")
        wt = spool.tile([P, 4, NTILE, K2], F32, tag="wt", name="wt")
        idxI = spool.tile([P, 16, GCOL], I16, tag="idxI", name="idxI")
        idxC = spool.tile([K2 * NGRP, 16, GCOL], I16, tag="idxC", name="idxC")
        twrap = spool.tile([P, NGRP * K2 * GCOL], I16, tag="twrap", name="twrap")

        nc.sync.dma_start(xc_sb[:], xc)
        nc.sync.dma_start(wofl_sb[:], wofl)
        nc.sync.dma_start(wdcl_sb[:], wdcl)
        nc.sync.dma_start(boff_sb[:], boff)
        nc.sync.dma_start(bdc_sb[:], bdc)
        nc.sync.dma_start(pyb_sb[:], pyb)
        nc.sync.dma_start(pxb_sb[:], pxb)
        nc.sync.dma_start(pypxbT_sb[:], pypxbT)

        # ---- A: offset conv, 12 sub-chunks of 384 cols; out regrouped ----
        with tc.tile_pool(name="psA", bufs=2, space="PSUM") as psa:
            for sc in range(NSUB):
                g, sub = sc // 4, sc % 4
                ncols = CONV_ROWS * W  # 384
                ps = psa.tile([18, ncols], F32, tag="psoff", name="psoff")
                n_mm = 2 * K2
                mm = 0
                xcf = xc_sb[:]
                for chalf in range(2):
                    for tap in range(K2):
                        ti, tj = tap // K, tap % K
                        rhs = dataclasses.replace(
                            xcf,
                            ap=[
                                [xcf.ap[0][0], P],
                                [PADW, CONV_ROWS],
                                [1, W],
                            ],
                            offset=xcf.offset
                            + chalf * (PADH * PADW)
                            + ((sc * CONV_ROWS + ti) * PADW + tj),
                        )
                        nc.tensor.matmul(
                            ps[:],
                            wofl_sb[:, chalf, tap],
                            rhs,
                            start=(mm == 0),
                            stop=(mm == n_mm - 1),
                        )
                        mm += 1
                # group g at partitions [g*32, g*32+18):
                # rows +0..8 = dy taps, +9..17 = dx taps
                nc.scalar.activation(
                    offb[g * 32 : g * 32 + 18, sub * ncols : (sub + 1) * ncols],
                    ps[:],
                    Act.Identity,
                    bias=boff_sb[:],
                )

        # ---- C2: tap-major index math ([128, 1536]) ----
        with tc.tile_pool(name="scr2", bufs=1) as scr2:
            pypx = scr2.tile([P, GP], F32, tag="pypx", name="pypx")
            nc.vector.tensor_tensor(pypx[:], offb[:], pypxbT_sb[:], Op.add)
            nc.vector.tensor_scalar(pypx[:], pypx[:], -0.4999999, None, Op.add)
            nc.vector.tensor_scalar(
                pypx[:], pypx[:], MAGIC, -MAGIC, Op.add, Op.add
            )
            nc.vector.tensor_scalar(
                pypx[:], pypx[:], 16.0, 111.0, Op.max, Op.min
            )  # quad anchor (shifted +16)
            # partition-shift by 9 via DMA so the y*96+x mix is base-aligned
            anchX = scr2.tile([P, GP], F32, tag="anchX", name="anchX")
            nc.sync.dma_start(anchX[0:119, :], pypx[9:128, :])
            idxf = scr2.tile([P, GP], F32, tag="idxf", name="idxf")
            nc.vector.scalar_tensor_tensor(
                idxf[0:74, :], pypx[0:74, :], 96.0, anchX[0:74, :],
                Op.mult, Op.add,
            )
            nc.vector.tensor_scalar(
                idxf[0:74, :], idxf[0:74, :], -1552.0, None, Op.add
            )
            # convert + restride: idxI[g*32+tap, r, col] <- idxf[g*32+tap, col*16+r]
            for g in range(NGRP):
                src = idxf[g * 32 : g * 32 + 9, :]
                src = dataclasses.replace(
                    src, ap=[src.ap[0], [1, 16], [16, GCOL]]
                )
                nc.vector.tensor_copy(
                    idxI[g * 32 : g * 32 + 9, :, :], src
                )

        # ---- D: fold into wrapped-16 layout (contiguous DMAs) ----
        # hop 1: compact to contiguous rows: idxC[g*9+tap, r, col]
        for g in range(NGRP):
            nc.sync.dma_start(
                idxC[g * K2 : (g + 1) * K2, :, :],
                idxI[g * 32 : g * 32 + K2, :, :],
            )
        # hop 2: twrap[r, (g*9+tap)*GCOL + col] <- idxC[g*9+tap, r, col]
        cfl = idxC[:]
        pitch_c = cfl.ap[0][0]
        for r in range(16):
            src = dataclasses.replace(
                cfl,
                ap=[[pitch_c, K2 * NGRP], [1, GCOL]],
                offset=cfl.offset + r * GCOL,
            )
            nc.sync.dma_start(twrap[r : r + 1, :], src)
        for g in range(1, 8):
            nc.sync.dma_start(twrap[16 * g : 16 * (g + 1), :], twrap[0:16, :])

        # ---- B: transpose offsets to pixel layout doff[p, t, (dy9,dx9)] ----
        with tc.tile_pool(name="psB", bufs=4, space="PSUM") as psb:
            for tl in range(GTL):
                pt = psb.tile([P, P], F32, tag="pofft", name="pofft")
                nc.tensor.transpose(
                    pt[:], offb[:, tl * P : (tl + 1) * P], ident_f[:]
                )
                for g in range(NGRP):
                    nc.scalar.copy(
                        doff[:, g * GTL + tl, :],
                        pt[:, g * 32 : g * 32 + 18],
                    )

        # ---- C1: pixel-major weight math (DVE over [128, 36*9]) ----
        with tc.tile_pool(name="scr", bufs=1) as scr:
            sh = [P, NTILE, K2]

            def tmp(tag):
                return scr.tile(sh, F32, tag=tag, name=tag)

            dy = doff[:, :, 0:9]
            dx = doff[:, :, 9:18]
            py = tmp("py")
            px = tmp("px")
            # pyb/pxb carry the +16 shift already
            nc.vector.tensor_tensor(py[:], pyb_sb[:], dy, Op.add)
            nc.vector.tensor_tensor(px[:], pxb_sb[:], dx, Op.add)
            y0 = tmp("y0")
            x0 = tmp("x0")
            nc.vector.tensor_scalar(y0[:], py[:], -0.4999999, None, Op.add)
            nc.vector.tensor_scalar(y0[:], y0[:], MAGIC, -MAGIC, Op.add, Op.add)
            nc.vector.tensor_scalar(x0[:], px[:], -0.4999999, None, Op.add)
            nc.vector.tensor_scalar(x0[:], x0[:], MAGIC, -MAGIC, Op.add, Op.add)
            ly = tmp("ly")
            lx = tmp("lx")
            nc.vector.tensor_tensor(ly[:], py[:], y0[:], Op.subtract)
            nc.vector.tensor_tensor(lx[:], px[:], x0[:], Op.subtract)

            ta_ = tmp("ta")
            tb_ = tmp("tb")
            tc_ = tmp("tc")
            td_ = tmp("td")

            def wab(f0, frac, wA, wB):
                # slot weights for a quad axis (coords shifted +16):
                # wA = (1-frac)*[16<=f0<=111] + frac*[f0==15]
                # wB = frac*[16<=f0<=110]
                nc.vector.tensor_scalar(ta_[:], f0[:], 16.0, None, Op.is_ge)
                nc.vector.tensor_scalar(tb_[:], f0[:], 111.0, None, Op.is_le)
                nc.vector.tensor_tensor(tb_[:], ta_[:], tb_[:], Op.mult)
                nc.vector.tensor_scalar(tc_[:], frac[:], -1.0, 1.0, Op.mult, Op.add)
                nc.vector.tensor_tensor(tb_[:], tc_[:], tb_[:], Op.mult)
                nc.vector.tensor_scalar(td_[:], f0[:], 15.0, None, Op.is_equal)
                nc.vector.tensor_tensor(td_[:], frac[:], td_[:], Op.mult)
                nc.vector.tensor_tensor(wA[:], tb_[:], td_[:], Op.add)
                nc.vector.tensor_scalar(tc_[:], f0[:], 110.0, None, Op.is_le)
                nc.vector.tensor_tensor(tc_[:], ta_[:], tc_[:], Op.mult)
                nc.vector.tensor_tensor(wB[:], frac[:], tc_[:], Op.mult)

            wyA = tmp("wyA")
            wyB = tmp("wyB")
            wxA = tmp("wxA")
            wxB = tmp("wxB")
            wab(y0, ly, wyA, wyB)
            wab(x0, lx, wxA, wxB)
            # corner cr = b*2 + a (b = x entry, a = y row): weight wy_a * wx_b
            nc.vector.tensor_tensor(wt[:, 0], wyA[:], wxA[:], Op.mult)
            nc.vector.tensor_tensor(wt[:, 1], wyB[:], wxA[:], Op.mult)
            nc.vector.tensor_tensor(wt[:, 2], wyA[:], wxB[:], Op.mult)
            nc.vector.tensor_tensor(wt[:, 3], wyB[:], wxB[:], Op.mult)

        # ---- E: main loop ----
        with (
            tc.tile_pool(name="gpool", bufs=4) as gpool,
            tc.tile_pool(name="vpool", bufs=4) as vpool,
            tc.tile_pool(name="upool", bufs=8) as upool,
            tc.tile_pool(name="rpool", bufs=2) as rpool,
            tc.tile_pool(name="opool", bufs=2) as opool,
            tc.tile_pool(name="psT", bufs=4, space="PSUM") as pst,
            tc.tile_pool(name="psO", bufs=2, space="PSUM") as pso,
        ):
            # quad-table window view: [NQENT-1 entries, 1024] stride 512
            qt_win = dataclasses.replace(
                qt, ap=[[2 * C, NQENT - 1], [1, 4 * C]], offset=0
            )
            for s in range(NSTAGE):
                po = pso.tile([P, 2, SPX], F32, tag="po", name="po")
                for tap in range(K2):
                    g_t = gpool.tile([P, TPS, 4 * C], BF, tag="g", name="g")
                    gq, qq = s // 3, s % 3
                    base = (gq * K2 + tap) * GCOL + qq * SCOL
                    idxs = twrap[:, base : base + SCOL]
                    nc.gpsimd.dma_gather(
                        g_t[:],
                        qt_win,
                        idxs,
                        SPX,
                        SPX,
                        elem_size=4 * C,
                        elem_step=2 * C,
                        queue_num=(tap % NSWQ),
                    )
                    rst = rpool.tile([P, 2, SPX], BF, tag="rst", name="rst")
                    us = []
                    for tl in range(TPS):
                        t = s * TPS + tl
                        u = upool.tile([P, C], BF, tag="u", name="u")
                        w0 = wt[:, 0, t, tap : tap + 1]
                        # corner 0 on ScalarE (out = in*scale), rest on DVE;
                        # all four hoisted so ScalarE runs ahead of the DVE chain
                        nc.scalar.activation(
                            u[:], g_t[:, tl, 0:256], Act.Identity, scale=w0
                        )
                        us.append(u)
                    for tl in range(TPS):
                        t = s * TPS + tl
                        v = vpool.tile([P, C], BF, tag="v", name="v")
                        u = us[tl]
                        w1 = wt[:, 1, t, tap : tap + 1]
                        w2 = wt[:, 2, t, tap : tap + 1]
                        w3 = wt[:, 3, t, tap : tap + 1]
                        nc.vector.scalar_tensor_tensor(
                            v[:], g_t[:, tl, 256:512], w1, u[:], Op.mult, Op.add
                        )
                        nc.vector.scalar_tensor_tensor(
                            v[:], g_t[:, tl, 512:768], w2, v[:], Op.mult, Op.add
                        )
                        nc.vector.scalar_tensor_tensor(
                            v[:], g_t[:, tl, 768:1024], w3, v[:], Op.mult, Op.add
                        )
                        ptr = pst.tile([P, 2, P], BF, tag="ptr", name="ptr")
                        nc.tensor.transpose(
                            ptr[:, 0, :], v[:, 0:P], ident_bf[:]
                        )
                        nc.tensor.transpose(
                            ptr[:, 1, :], v[:, P : 2 * P], ident_bf[:]
                        )
                        nc.scalar.copy(
                            rst[:, :, tl * P : (tl + 1) * P], ptr[:]
                        )
                    for chalf in range(2):
                        for oh in range(2):
                            nc.tensor.matmul(
                                po[:, oh],
                                wdcl_sb[:, tap, chalf, oh],
                                rst[:, chalf],
                                start=(tap == 0 and chalf == 0),
                                stop=(tap == K2 - 1 and chalf == 1),
                            )
                for oh in range(2):
                    ob = opool.tile([P, SPX], F32, tag="ob", name="ob")
                    nc.scalar.activation(
                        ob[:], po[:, oh], Act.Identity, bias=bdc_sb[:, oh : oh + 1]
                    )
                    nc.sync.dma_start(
                        out[oh, :, s * SPX : (s + 1) * SPX], ob[:]
                    )


def _build():
    if "nc" in _BUILT:
        return _BUILT["nc"]
    nc = bacc.Bacc(
        "TRN2",
        target_bir_lowering=False,
        debug=False,
        enable_asserts=False,
        num_devices=NCORES,
        num_swdge_queues=NSWQ,
    )
    qt = nc.dram_tensor("qt", [NQENT, 2 * C], BF, kind="ExternalInput").ap()
    xc = nc.dram_tensor("xc", [P, 2, PADH * PADW], BF, kind="ExternalInput").ap()
    wofl = nc.dram_tensor("wofl", [P, 2, K2, 18], BF, kind="ExternalInput").ap()
    boff = nc.dram_tensor("boff", [18, 1], F32, kind="ExternalInput").ap()
    wdcl = nc.dram_tensor("wdcl", [P, K2, 2, 2, P], BF, kind="ExternalInput").ap()
    bdc = nc.dram_tensor("bdc", [P, 2], F32, kind="ExternalInput").ap()
    pyb = nc.dram_tensor("pyb", [P, NTILE, K2], F32, kind="ExternalInput").ap()
    pxb = nc.dram_tensor("pxb", [P, NTILE, K2], F32, kind="ExternalInput").ap()
    pypxbT = nc.dram_tensor("pypxbT", [P, GP], F32, kind="ExternalInput").ap()
    out = nc.dram_tensor("out", [2, P, NPIX], F32, kind="ExternalOutput").ap()
    with tile.TileContext(nc) as tc:
        _emit(tc, nc, (qt, xc, wofl, boff, wdcl, bdc, pyb, pxb, pypxbT, out))
    nc.compile()
    _BUILT["nc"] = nc
    return nc


def _prep_core(k, x, w_off, b_off, w_dc, b_dc):
    b, half = k // 2, k % 2
    y0 = half * ROWS
    xs = x[b]  # [C,H,W] f32
    qt = np.zeros((NQENT, 2 * C), np.float32)
    qt[:NTOK, 0:C] = xs.transpose(1, 2, 0).reshape(NTOK, C)
    qt[: (H - 1) * W, C : 2 * C] = xs[:, 1:, :].transpose(1, 2, 0).reshape(
        (H - 1) * W, C
    )
    xc = np.zeros((C, PADH, PADW), np.float32)
    r0, r1 = max(0, y0 - 1), min(H, y0 + ROWS + 1)
    xc[:, (r0 - (y0 - 1)) : (r1 - (y0 - 1)), 1 : 1 + W] = xs[:, r0:r1, :]
    xc = xc.reshape(2, P, PADH * PADW).transpose(1, 0, 2)

    # conv output channels reordered to [dy0..8, dx0..8]
    perm = np.concatenate([np.arange(0, 18, 2), np.arange(1, 18, 2)])
    wofl = (
        w_off.reshape(2 * K2, 2, P, K2)   # [oc, chalf, c, tap]
        .transpose(2, 1, 3, 0)            # [c, chalf, tap, oc]
        [:, :, :, perm]
        .copy()
    )
    boff = b_off[perm].reshape(18, 1).astype(np.float32)
    wdcl = (
        w_dc.reshape(2, P, 2, P, K2)      # [oh, o, chalf, c, tap]
        .transpose(3, 4, 2, 0, 1)         # [c, tap, chalf, oh, o]
        .copy()
    )
    bdc = b_dc.reshape(2, P).transpose(1, 0).copy()

    pp = np.arange(NPIX)
    yg = y0 + pp // W
    xg = pp % W
    ti = (np.arange(K2) // K)[None, :]
    tj = (np.arange(K2) % K)[None, :]
    pyb = (yg[:, None] - 1 + ti + 16.0).astype(np.float32).reshape(NTILE, P, K2)
    pxb = (xg[:, None] - 1 + tj + 16.0).astype(np.float32).reshape(NTILE, P, K2)

    # tap-major base table [128, 1152]: row g*32 + yx*9 + tap
    pypxbT = np.zeros((P, GP), np.float32)
    fo = np.arange(GP)
    for g in range(NGRP):
        p = g * GP + fo
        for tap in range(K2):
            pypxbT[g * 32 + tap] = (y0 + p // W) - 1 + tap // K + 16.0
            pypxbT[g * 32 + 9 + tap] = (p % W) - 1 + tap % K + 16.0

    import ml_dtypes

    bf16 = ml_dtypes.bfloat16
    return {
        "qt": qt.astype(bf16),
        "xc": xc.astype(bf16),
        "wofl": wofl.astype(bf16),
        "boff": boff,
        "wdcl": wdcl.astype(bf16),
        "bdc": bdc.astype(np.float32),
        "pyb": pyb.transpose(1, 0, 2).copy(),
        "pxb": pxb.transpose(1, 0, 2).copy(),
        "pypxbT": pypxbT,
    }


def kernel(x, w_off, b_off, w_dc, b_dc, _trace=False):
    nc = _build()
    x = np.asarray(x, np.float32)
    w_off = np.asarray(w_off, np.float32)
    b_off = np.asarray(b_off, np.float32)
    w_dc = np.asarray(w_dc, np.float32)
    b_dc = np.asarray(b_dc, np.float32)
    in_maps = [
        _prep_core(k, x, w_off, b_off, w_dc, b_dc) for k in range(NCORES)
    ]
    res = bass_utils.run_bass_kernel_spmd(
        nc, in_maps, core_ids=list(range(NCORES)), trace=_trace
    )
    out = np.empty((B, O, H, W), np.float32)
    for k in range(NCORES):
        b, half = k // 2, k % 2
        o = res.results[k]["out"]  # [2,128,4608]
        out[b, :, half * ROWS : (half + 1) * ROWS, :] = o.reshape(
            O, ROWS, W
        )
    if _trace:
        return out, res
    return out
